# revision 1
# baseline (speedup 1.0000x reference)
"""Trainium2 Bass kernel for DeformableTransformerEncoderLayer.

Sharding: 8 cores = (batch b in 0..3) x (half of the 8400 queries).
Each core:
  stage 1: value = src[b] @ val_w  -> DRAM val_cat, duplicated per parity so
           any 2-pixel bilinear window is one 256B-aligned 64-float slot.
  stage 2: per 128-query block: projections (PE), sampling-location math (DVE),
           dma_gather of 2x2 bilinear patches (2 slots/sample), weighted
           tree-fold combine (DVE), out-proj + LN + FFN + LN (PE/ACT/DVE).
"""

import sys
import os
import numpy as np
from contextlib import ExitStack

for _p in ("/root/.axon_site/_ro/trn_rl_repo", "/opt/trn_rl_repo"):
    if os.path.isdir(_p) and _p not in sys.path:
        sys.path.insert(0, _p)

import concourse.bass as bass
import concourse.bacc as bacc
import concourse.tile as tile
from concourse import mybir
from concourse.bass_utils import run_bass_kernel_spmd

dt = mybir.dt
Alu = mybir.AluOpType
ActF = mybir.ActivationFunctionType
AX = mybir.AxisListType

# ---------------------------------------------------------------- problem dims
B, Lq, DM, NH, LVL, PTS, DFF, HD = 4, 8400, 256, 8, 3, 4, 1024, 32
SHAPES = [(80, 80), (40, 40), (20, 20)]          # (H, W)
LSI = [0, 6400, 8000]
NCORES = 8
QPC = Lq // 2                                     # queries per core = 4200
NBLK = 33
QPAD = NBLK * 128                                 # 4224
NJ = NH * LVL * PTS                               # 96 sample coords
HPREG = 2 * 2 * 4200                              # slots per head-pair region
CAT_SLOTS = 4 * HPREG                             # 67200
ODD_OFF = 4200 * 64                               # elems: parity offset within head
HP_OFF = HPREG * 64                               # elems: region stride

_CACHE = {}
DEBUG = False


# ------------------------------------------------------------------ host prep
def _host_consts():
    j = np.arange(NJ)
    h = j // (LVL * PTS)
    l = (j % (LVL * PTS)) // PTS
    W = np.array([SHAPES[i][1] for i in range(LVL)], np.float32)[l]
    H = np.array([SHAPES[i][0] for i in range(LVL)], np.float32)[l]
    lsi = np.array(LSI, np.float32)[l]
    consts = np.stack([W, W - 2, H - 1, lsi,
                       ((h % 2) * 8400).astype(np.float32)], axis=1)
    consts = np.ascontiguousarray(consts.astype(np.float32))

    E3 = np.zeros((36, NJ), np.float32)
    E3[l, j] = 1.0
    E3[32 + l, j] = 1.0
    REP = np.zeros((16, 128), np.float32)
    REP[np.arange(128) % 16, np.arange(128)] = 1.0
    IDENT = np.eye(128, dtype=np.float32)
    scale_m = np.zeros((36, 1), np.float32)
    scale_m[0:3, 0] = [SHAPES[i][1] for i in range(LVL)]
    scale_m[32:35, 0] = [SHAPES[i][0] for i in range(LVL)]
    return consts, E3, REP, IDENT, scale_m


def _perm_off_w(off_w):
    cols = np.arange(NH * LVL * PTS * 2).reshape(NH, LVL, PTS, 2)
    return (np.ascontiguousarray(off_w[:, cols[..., 0].reshape(-1)]),
            np.ascontiguousarray(off_w[:, cols[..., 1].reshape(-1)]))


def _ktiles(w):
    K, N = w.shape
    return np.ascontiguousarray(w.reshape(K // 128, 128, N).astype(np.float32))


# -------------------------------------------------------------- device program
def _build_program():
    nc = bacc.Bacc("TRN2", target_bir_lowering=False, debug=False, num_swdge_queues=4)
    f32 = dt.float32

    src_full = nc.dram_tensor("src_full", [Lq, DM], f32, kind="ExternalInput")
    src_q = nc.dram_tensor("src_q", [QPAD, DM], f32, kind="ExternalInput")
    qpe_q = nc.dram_tensor("qpe_q", [QPAD, DM], f32, kind="ExternalInput")
    refs = nc.dram_tensor("refs", [6, QPAD], f32, kind="ExternalInput")
    w_in = {}
    for name, kt, n in (("val_w", 2, DM), ("off_wx", 2, NJ), ("off_wy", 2, NJ),
                        ("aw_w", 2, NJ), ("out_w", 2, DM), ("lin1_w", 2, DFF),
                        ("lin2_w", 8, DM)):
        w_in[name] = nc.dram_tensor(name, [kt, 128, n], f32, kind="ExternalInput")
    consts = nc.dram_tensor("consts", [NJ, 5], f32, kind="ExternalInput")
    e3 = nc.dram_tensor("e3", [36, NJ], f32, kind="ExternalInput")
    rep = nc.dram_tensor("rep", [16, 128], f32, kind="ExternalInput")
    ident = nc.dram_tensor("ident", [128, 128], f32, kind="ExternalInput")
    scale_m = nc.dram_tensor("scale_m", [36, 1], f32, kind="ExternalInput")

    out_d = nc.dram_tensor("out", [QPC, DM], f32, kind="ExternalOutput")
    valcat = nc.dram_tensor("valcat", [CAT_SLOTS, 64], f32)
    dbg = {}
    if DEBUG:
        dbg["att"] = nc.dram_tensor("dbg_att", [QPAD, DM], f32, kind="ExternalOutput")
        dbg["slot0"] = nc.dram_tensor("dbg_slot0", [NBLK, NJ, 128], f32, kind="ExternalOutput")
        dbg["slot1"] = nc.dram_tensor("dbg_slot1", [NBLK, NJ, 128], f32, kind="ExternalOutput")
        dbg["wfull"] = nc.dram_tensor("dbg_wfull", [NBLK, 128, 384], f32, kind="ExternalOutput")
        dbg["asm"] = nc.dram_tensor("dbg_asm", [NBLK, 128, NJ], f32, kind="ExternalOutput")
        dbg["x"] = nc.dram_tensor("dbg_x", [NBLK, NJ, 128], f32, kind="ExternalOutput")
        dbg["y"] = nc.dram_tensor("dbg_y", [NBLK, NJ, 128], f32, kind="ExternalOutput")
        dbg["hn"] = nc.dram_tensor("dbg_hn", [QPAD, DM], f32, kind="ExternalOutput")
        for nm in ("fx", "fy", "wa", "wb", "wy0", "wy1", "vy1", "y0d", "x0d"):
            dbg[nm] = nc.dram_tensor("dbg_" + nm, [NBLK, NJ, 128], f32, kind="ExternalOutput")

    def ap(base, off, dims):
        return bass.AP(tensor=base.tensor, offset=base.offset + off,
                       ap=[list(d) for d in dims])

    with tile.TileContext(nc) as tc, ExitStack() as ctx:
        V, S, T, G = nc.vector, nc.scalar, nc.tensor, nc.gpsimd

        def stt(out, in0, scalar, in1, op0, op1):
            return V.scalar_tensor_tensor(out=out, in0=in0, scalar=scalar,
                                          in1=in1, op0=op0, op1=op1)

        wp = ctx.enter_context(tc.tile_pool(name="weights", bufs=1))
        w_sb = {}
        for name, t in w_in.items():
            kt, n = t.shape[0], t.shape[2]
            s = wp.tile([128, kt, n], f32, name=name + "_sb")
            nc.sync.dma_start(out=s[:], in_=t[:].rearrange("a p n -> p a n"))
            w_sb[name] = s
        consts_sb = wp.tile([NJ, 5], f32)
        nc.sync.dma_start(out=consts_sb[:], in_=consts[:])
        e3_sb = wp.tile([36, NJ], f32)
        nc.sync.dma_start(out=e3_sb[:], in_=e3[:])
        rep_sb = wp.tile([16, 128], f32)
        nc.sync.dma_start(out=rep_sb[:], in_=rep[:])
        id_sb = wp.tile([128, 128], f32)
        nc.sync.dma_start(out=id_sb[:], in_=ident[:])
        scm_sb = wp.tile([36, 1], f32)
        nc.sync.dma_start(out=scm_sb[:], in_=scale_m[:])
        eps_sb = wp.tile([128, 1], f32)
        V.memset(eps_sb[:], 1e-5)
        zpad = wp.tile([8, 64], f32)
        V.memset(zpad[:], 0.0)
        # last odd slot per head (pixels 8399..8400) is never stored; zero it
        nc.sync.dma_start(
            out=ap(valcat[:], ODD_OFF + 4199 * 64, [[2 * 4200 * 64, 8], [1, 64]]),
            in_=zpad[:])

        cW = consts_sb[:, 0:1]
        cWm2 = consts_sb[:, 1:2]
        cHm1 = consts_sb[:, 2:3]
        cLsi = consts_sb[:, 3:4]
        cHH = consts_sb[:, 4:5]

        pps = ctx.enter_context(tc.tile_pool(name="pps", bufs=2, space="PSUM"))

        def psum(shape, tag):
            return pps.tile(shape, f32, tag=tag, name=tag)

        # ------------------------------------------------------------ stage 1
        with tc.tile_pool(name="s1", bufs=2) as s1:
            NT1 = (Lq + 127) // 128  # 66
            for it in range(NT1):
                n = min(128, Lq - it * 128)
                st = s1.tile([128, DM], f32, tag="st")
                nc.sync.dma_start(out=st[:n], in_=src_full[it * 128: it * 128 + n])
                sT = s1.tile([128, 2, 128], f32, tag="sT")
                for kt in range(2):
                    pt = psum([128, 128], "tr")
                    T.transpose(out=pt[:, :n], in_=st[:n, kt * 128:(kt + 1) * 128],
                                identity=id_sb[:n, :n])
                    S.copy(out=sT[:, kt, :n], in_=pt[:, :n])
                vps = psum([128, DM], "mm")
                T.matmul(vps[:n], lhsT=sT[:, 0, :n], rhs=w_sb["val_w"][:, 0, :],
                         start=True, stop=False)
                T.matmul(vps[:n], lhsT=sT[:, 1, :n], rhs=w_sb["val_w"][:, 1, :],
                         start=False, stop=True)
                vsb = s1.tile([128, DM], f32, tag="vsb")
                S.copy(out=vsb[:n], in_=vps[:n])
                HSTR = 2 * 4200 * 64  # head stride: [even 4200][odd 4200] slots
                dst_e = ap(valcat[:], it * 128 * 32,
                           [[32, n], [HSTR, 8], [1, 32]])
                nc.sync.dma_start(
                    out=dst_e,
                    in_=vsb[:n].rearrange("t (a d) -> t a d", a=8))
                if it == 0:
                    dst_o = ap(valcat[:], ODD_OFF,
                               [[32, n - 1], [HSTR, 8], [1, 32]])
                    nc.sync.dma_start(
                        out=dst_o,
                        in_=vsb[1:n].rearrange("t (a d) -> t a d", a=8))
                else:
                    dst_o = ap(valcat[:], ODD_OFF + (it * 128 - 1) * 32,
                               [[32, n], [HSTR, 8], [1, 32]])
                    nc.sync.dma_start(
                        out=dst_o,
                        in_=vsb[:n].rearrange("t (a d) -> t a d", a=8))

        # ------------------------------------------------------------ stage 2
        sp = ctx.enter_context(tc.tile_pool(name="sp", bufs=2))
        sc = ctx.enter_context(tc.tile_pool(name="sc", bufs=1))
        mp = ctx.enter_context(tc.tile_pool(name="mp", bufs=2))
        gp = ctx.enter_context(tc.tile_pool(name="gp", bufs=2))

        RNE = 12582912.0  # 1.5 * 2^23: RNE stays in unit-spacing range
        J = NJ

        for ib in range(NBLK):
            q0 = ib * 128
            sq = sp.tile([128, DM], f32, tag="sq")
            nc.sync.dma_start(out=sq[:], in_=src_q[q0:q0 + 128])
            qp = sp.tile([128, DM], f32, tag="qp")
            nc.sync.dma_start(out=qp[:], in_=qpe_q[q0:q0 + 128])
            qt = sp.tile([128, DM], f32, tag="qt")
            stt(qt[:], sq[:], 0.0, qp[:], Alu.bypass, Alu.add)
            qT = sp.tile([128, 2, 128], f32, tag="qT")
            for kt in range(2):
                pt = psum([128, 128], "tr")
                T.transpose(out=pt[:], in_=qt[:, kt * 128:(kt + 1) * 128],
                            identity=id_sb[:])
                S.copy(out=qT[:, kt, :], in_=pt[:])
            rf = sc.tile([36, 128], f32, tag="rf")
            nc.sync.dma_start(out=rf[0:3], in_=refs[0:3, q0:q0 + 128])
            nc.sync.dma_start(out=rf[32:35], in_=refs[3:6, q0:q0 + 128])
            rw = sc.tile([36, 128], f32, tag="rw")
            V.tensor_scalar(out=rw[0:3], in0=rf[0:3], scalar1=scm_sb[0:3],
                            scalar2=-0.5, op0=Alu.mult, op1=Alu.add)
            V.tensor_scalar(out=rw[32:35], in0=rf[32:35], scalar1=scm_sb[32:35],
                            scalar2=-0.5, op0=Alu.mult, op1=Alu.add)

            xy = {}
            for name, wkey, r0, r1 in (("x", "off_wx", 0, 3), ("y", "off_wy", 32, 35)):
                pxy = psum([128, 128], "mm")
                T.matmul(pxy[:J], lhsT=w_sb[wkey][:, 0, :], rhs=qT[:, 0, :],
                         start=True, stop=False)
                T.matmul(pxy[:J], lhsT=w_sb[wkey][:, 1, :], rhs=qT[:, 1, :],
                         start=False, stop=False)
                T.matmul(pxy[:J], lhsT=e3_sb[r0:r1, :], rhs=rw[r0:r1, :],
                         start=False, stop=True)
                xs = sc.tile([128, 128], f32, tag="xy" + name)
                S.copy(out=xs[:J], in_=pxy[:J])
                xy[name] = xs
            x_sb, y_sb = xy["x"], xy["y"]

            def nt(tag):
                return sc.tile([128, 128], f32, tag=tag, name=tag)

            def floor_(src_t, tag):
                a = nt(tag + "a")
                V.tensor_scalar(out=a[:J], in0=src_t[:J], scalar1=RNE,
                                scalar2=-RNE, op0=Alu.add, op1=Alu.add)
                g_ = nt(tag + "g")
                stt(g_[:J], a[:J], 0.0, src_t[:J], Alu.bypass, Alu.is_gt)
                f = nt(tag + "f")
                stt(f[:J], a[:J], 0.0, g_[:J], Alu.bypass, Alu.subtract)
                return f

            x0 = floor_(x_sb, "x0")
            fx = nt("fx")
            stt(fx[:J], x_sb[:J], 0.0, x0[:J], Alu.bypass, Alu.subtract)
            y0 = floor_(y_sb, "y0")
            fy = nt("fy")
            stt(fy[:J], y_sb[:J], 0.0, y0[:J], Alu.bypass, Alu.subtract)

            xc = nt("xc")
            V.tensor_scalar(out=xc[:J], in0=x0[:J], scalar1=0.0, scalar2=cWm2,
                            op0=Alu.max, op1=Alu.min)
            eqA0 = nt("eqA0")
            stt(eqA0[:J], x0[:J], 0.0, xc[:J], Alu.bypass, Alu.is_equal)
            eqA1 = nt("eqA1")
            stt(eqA1[:J], x0[:J], 1.0, xc[:J], Alu.add, Alu.is_equal)
            eqB0 = nt("eqB0")
            stt(eqB0[:J], x0[:J], -1.0, xc[:J], Alu.add, Alu.is_equal)
            dA = nt("dA")
            stt(dA[:J], eqA1[:J], 0.0, eqA0[:J], Alu.bypass, Alu.subtract)
            wA = nt("wA")
            stt(wA[:J], fx[:J], 0.0, dA[:J], Alu.bypass, Alu.mult)
            stt(wA[:J], wA[:J], 0.0, eqA0[:J], Alu.bypass, Alu.add)
            dB = nt("dB")
            stt(dB[:J], eqA0[:J], 0.0, eqB0[:J], Alu.bypass, Alu.subtract)
            wB = nt("wB")
            stt(wB[:J], fx[:J], 0.0, dB[:J], Alu.bypass, Alu.mult)
            stt(wB[:J], wB[:J], 0.0, eqB0[:J], Alu.bypass, Alu.add)

            yr0 = nt("yr0")
            V.tensor_scalar(out=yr0[:J], in0=y0[:J], scalar1=0.0, scalar2=cHm1,
                            op0=Alu.max, op1=Alu.min)
            y1 = nt("y1")
            V.tensor_scalar(out=y1[:J], in0=y0[:J], scalar1=1.0, op0=Alu.add, scalar2=None)
            yr1 = nt("yr1")
            V.tensor_scalar(out=yr1[:J], in0=y1[:J], scalar1=0.0, scalar2=cHm1,
                            op0=Alu.max, op1=Alu.min)
            vy0 = nt("vy0")
            stt(vy0[:J], y0[:J], 0.0, yr0[:J], Alu.bypass, Alu.is_equal)
            wy0 = nt("wy0")
            stt(wy0[:J], fy[:J], 0.0, vy0[:J], Alu.bypass, Alu.mult)
            stt(wy0[:J], vy0[:J], 0.0, wy0[:J], Alu.bypass, Alu.subtract)
            vy1 = nt("vy1")
            stt(vy1[:J], y1[:J], 0.0, yr1[:J], Alu.bypass, Alu.is_equal)
            wy1 = nt("wy1")
            stt(wy1[:J], fy[:J], 0.0, vy1[:J], Alu.bypass, Alu.mult)

            slots = []
            for r, yr in ((0, yr0), (1, yr1)):
                u = nt(f"u{r}")
                stt(u[:J], yr[:J], cW, xc[:J], Alu.mult, Alu.add)
                ps_ = nt(f"ps{r}")
                V.tensor_scalar(out=ps_[:J], in0=u[:J], scalar1=cLsi, op0=Alu.add, scalar2=None)
                halfp = nt(f"hp{r}")
                V.tensor_scalar(out=halfp[:J], in0=ps_[:J], scalar1=0.5,
                                op0=Alu.mult, scalar2=None)
                fh = floor_(halfp, f"fh{r}")
                pi = nt(f"pi{r}")
                stt(pi[:J], fh[:J], -2.0, ps_[:J], Alu.mult, Alu.add)
                v = nt(f"v{r}")
                V.tensor_scalar(out=v[:J], in0=pi[:J], scalar1=4199.5,
                                scalar2=cHH, op0=Alu.mult, op1=Alu.add)
                sl = mp.tile([128, 128], f32, tag=f"slot{r}", name=f"slot{r}")
                stt(sl[:J], ps_[:J], 0.5, v[:J], Alu.mult, Alu.add)
                slots.append(sl)

            # wrapped idx [16, 1536] -> replicate to [128, 1536] -> int16
            wf16 = mp.tile([16, 1536], f32, tag="wf16")
            for qb in range(8):
                for r in (0, 1):
                    pt = psum([128, 128], "tr")
                    T.transpose(out=pt[:16, :J],
                                in_=slots[r][:J, qb * 16:(qb + 1) * 16],
                                identity=id_sb[:J, :J])
                    dst = ap(wf16[:], r * 96 + qb, [[1536, 16], [192, 8], [8, 12]])
                    V.tensor_copy(out=dst, in_=pt[:16, :J].rearrange(
                        "p (h m) -> p h m", h=8))
            idxw = mp.tile([128, 1536], dt.int16, tag="idxw")
            for seg in range(3):
                pr = psum([128, 512], "w512")
                T.matmul(pr[:], lhsT=rep_sb[:],
                         rhs=wf16[:, seg * 512:(seg + 1) * 512],
                         start=True, stop=True)
                V.tensor_copy(out=idxw[:, seg * 512:(seg + 1) * 512], in_=pr[:])

            # attention weight softmax
            awp = psum([128, 128], "mm")
            T.matmul(awp[:, :J], lhsT=qT[:, 0, :], rhs=w_sb["aw_w"][:, 0, :],
                     start=True, stop=False)
            T.matmul(awp[:, :J], lhsT=qT[:, 1, :], rhs=w_sb["aw_w"][:, 1, :],
                     start=False, stop=True)
            aw_sb = sc.tile([128, NJ], f32, tag="aw_sb")
            S.copy(out=aw_sb[:], in_=awp[:, :J])
            rmax = sc.tile([128, 8], f32, tag="rmax")
            V.tensor_reduce(out=rmax[:],
                            in_=aw_sb[:].rearrange("p (h m) -> p h m", h=8),
                            axis=AX.X, op=Alu.max)
            xm = sc.tile([128, NJ], f32, tag="xm")
            stt(xm[:], aw_sb[:], 0.0,
                ap(rmax[:], 0, [[8, 128], [1, 8], [0, 12]]),
                Alu.bypass, Alu.subtract)
            exw = sc.tile([128, NJ], f32, tag="exw")
            S.activation(out=exw[:], in_=xm[:], func=ActF.Exp)
            ssum = sc.tile([128, 8], f32, tag="ssum")
            V.tensor_reduce(out=ssum[:],
                            in_=exw[:].rearrange("p (h m) -> p h m", h=8),
                            axis=AX.X, op=Alu.add)
            rec = sc.tile([128, 8], f32, tag="rec")
            V.reciprocal(out=rec[:], in_=ssum[:])
            asm = mp.tile([128, NJ], f32, tag="asm")
            stt(asm[:], exw[:], 0.0,
                ap(rec[:], 0, [[8, 128], [1, 8], [0, 12]]),
                Alu.bypass, Alu.mult)

            # token-major combine weights
            wxp = mp.tile([128, 192], f32, tag="wxp")
            wyp = mp.tile([128, 192], f32, tag="wyp")
            for src_t, dstt, col in ((wA, wxp, 0), (wB, wxp, 96),
                                     (wy0, wyp, 0), (wy1, wyp, 96)):
                pt = psum([128, 128], "tr")
                T.transpose(out=pt[:, :J], in_=src_t[:J, :],
                            identity=id_sb[:J, :J])
                S.copy(out=dstt[:, col:col + 96], in_=pt[:, :J])
            wrow = mp.tile([128, 192], f32, tag="wrow")
            stt(wrow[:], wyp[:], 0.0,
                ap(asm[:], 0, [[96, 128], [0, 2], [1, 96]]), Alu.bypass, Alu.mult)
            wfull = mp.tile([128, 384], f32, tag="wfull")
            for r in (0, 1):
                stt(ap(wfull[:], r * 24, [[384, 128], [48, 8], [2, 12], [1, 2]]),
                    ap(wrow[:], r * 96, [[192, 128], [12, 8], [1, 12], [0, 2]]),
                    0.0,
                    ap(wxp[:], 0, [[192, 128], [12, 8], [1, 12], [96, 2]]),
                    Alu.bypass, Alu.mult)

            # gather + combine per head-pair
            att = mp.tile([128, DM], f32, tag="att")
            for hp in range(4):
                g = gp.tile([128, 48 * 64], f32, tag="g")
                G.dma_gather(
                    out_ap=g[:].rearrange("p (j e) -> p j e", e=64),
                    in_ap=ap(valcat[:], hp * HP_OFF, [[64, HPREG], [1, 64]]),
                    idxs_ap=idxw[:, hp * 384:(hp + 1) * 384],
                    num_idxs=6144, num_idxs_reg=6144,
                    elem_size=64, elem_step=64, single_packet=False,
                    queue_num=hp)
                wg = gp.tile([128, 48 * 64], f32, tag="wg")
                stt(wg[:].rearrange("p (j c d) -> p j c d", c=2, d=32),
                    g[:].rearrange("p (j c d) -> p j c d", c=2, d=32), 0.0,
                    ap(wfull[:], hp * 96, [[384, 128], [2, 48], [1, 2], [0, 32]]),
                    Alu.bypass, Alu.mult)
                f1 = gp.tile([128, 24 * 64], f32, tag="f1")
                stt(f1[:], ap(wg[:], 0, [[3072, 128], [128, 24], [1, 64]]), 0.0,
                    ap(wg[:], 64, [[3072, 128], [128, 24], [1, 64]]),
                    Alu.bypass, Alu.add)
                f2 = gp.tile([128, 12 * 64], f32, tag="f2")
                stt(f2[:], ap(f1[:], 0, [[1536, 128], [128, 12], [1, 64]]), 0.0,
                    ap(f1[:], 64, [[1536, 128], [128, 12], [1, 64]]),
                    Alu.bypass, Alu.add)
                t4 = gp.tile([128, 4 * 64], f32, tag="t4")
                stt(t4[:], ap(f2[:], 0, [[768, 128], [192, 4], [1, 64]]), 0.0,
                    ap(f2[:], 64, [[768, 128], [192, 4], [1, 64]]),
                    Alu.bypass, Alu.add)
                stt(t4[:], t4[:], 0.0,
                    ap(f2[:], 128, [[768, 128], [192, 4], [1, 64]]),
                    Alu.bypass, Alu.add)
                t5 = gp.tile([128, 2 * 64], f32, tag="t5")
                stt(t5[:], ap(t4[:], 0, [[256, 128], [128, 2], [1, 64]]), 0.0,
                    ap(t4[:], 64, [[256, 128], [128, 2], [1, 64]]),
                    Alu.bypass, Alu.add)
                stt(att[:, hp * 64:(hp + 1) * 64].rearrange(
                        "p (b d) -> p b d", d=32),
                    ap(t5[:], 0, [[128, 128], [64, 2], [1, 32]]), 0.0,
                    ap(t5[:], 32, [[128, 128], [64, 2], [1, 32]]),
                    Alu.bypass, Alu.add)

            if DEBUG:
                for nm, tt in (("fx", fx), ("fy", fy), ("wa", wA), ("wb", wB),
                               ("wy0", wy0), ("wy1", wy1), ("vy1", vy1),
                               ("y0d", y0), ("x0d", x0)):
                    nc.sync.dma_start(out=dbg[nm][ib], in_=tt[:NJ])
                nc.sync.dma_start(out=dbg["att"][q0:q0 + 128], in_=att[:])
                nc.sync.dma_start(out=dbg["slot0"][ib], in_=slots[0][:NJ])
                nc.sync.dma_start(out=dbg["slot1"][ib], in_=slots[1][:NJ])
                nc.sync.dma_start(out=dbg["wfull"][ib], in_=wfull[:])
                nc.sync.dma_start(out=dbg["asm"][ib], in_=asm[:])
                nc.sync.dma_start(out=dbg["x"][ib], in_=x_sb[:NJ])
                nc.sync.dma_start(out=dbg["y"][ib], in_=y_sb[:NJ])

            # out-proj + residual + LN1
            aT = sp.tile([128, 2, 128], f32, tag="aT")
            for kt in range(2):
                pt = psum([128, 128], "tr")
                T.transpose(out=pt[:], in_=att[:, kt * 128:(kt + 1) * 128],
                            identity=id_sb[:])
                S.copy(out=aT[:, kt, :], in_=pt[:])
            ops_ = psum([128, DM], "mm")
            T.matmul(ops_[:], lhsT=aT[:, 0, :], rhs=w_sb["out_w"][:, 0, :],
                     start=True, stop=False)
            T.matmul(ops_[:], lhsT=aT[:, 1, :], rhs=w_sb["out_w"][:, 1, :],
                     start=False, stop=True)

            def layernorm(src_ps, res_sb, tag):
                h1 = sc.tile([128, DM], f32, tag=tag + "h1")
                stt(h1[:], src_ps[:], 0.0, res_sb[:], Alu.bypass, Alu.add)
                mr = sc.tile([128, 1], f32, tag=tag + "mr")
                V.tensor_reduce(out=mr[:], in_=h1[:], axis=AX.X, op=Alu.add)
                m = sc.tile([128, 1], f32, tag=tag + "m")
                V.tensor_scalar(out=m[:], in0=mr[:], scalar1=1.0 / DM,
                                op0=Alu.mult, scalar2=None)
                d = sc.tile([128, DM], f32, tag=tag + "d")
                stt(d[:], h1[:], 0.0, ap(m[:], 0, [[1, 128], [0, DM]]),
                    Alu.bypass, Alu.subtract)
                sq2 = sc.tile([128, DM], f32, tag=tag + "sq")
                S.activation(out=sq2[:], in_=d[:], func=ActF.Square)
                vr = sc.tile([128, 1], f32, tag=tag + "vr")
                V.tensor_reduce(out=vr[:], in_=sq2[:], axis=AX.X, op=Alu.add)
                sd = sc.tile([128, 1], f32, tag=tag + "sd")
                S.activation(out=sd[:], in_=vr[:], func=ActF.Sqrt,
                             scale=1.0 / DM, bias=eps_sb[:])
                rstd = sc.tile([128, 1], f32, tag=tag + "rs")
                V.reciprocal(out=rstd[:], in_=sd[:])
                o = sp.tile([128, DM], f32, tag=tag + "o")
                V.tensor_scalar(out=o[:], in0=d[:], scalar1=rstd[:],
                                op0=Alu.mult, scalar2=None)
                return o

            hn = layernorm(ops_, sq, "ln1")

            if DEBUG:
                nc.sync.dma_start(out=dbg["hn"][q0:q0 + 128], in_=hn[:])
            hT = sp.tile([128, 2, 128], f32, tag="hT")
            for kt in range(2):
                pt = psum([128, 128], "tr")
                T.transpose(out=pt[:], in_=hn[:, kt * 128:(kt + 1) * 128],
                            identity=id_sb[:])
                S.copy(out=hT[:, kt, :], in_=pt[:])
            gsb = sp.tile([128, DFF], f32, tag="gsb")
            for nb in range(2):
                fps = psum([128, 512], "w512")
                T.matmul(fps[:], lhsT=hT[:, 0, :],
                         rhs=w_sb["lin1_w"][:, 0, nb * 512:(nb + 1) * 512],
                         start=True, stop=False)
                T.matmul(fps[:], lhsT=hT[:, 1, :],
                         rhs=w_sb["lin1_w"][:, 1, nb * 512:(nb + 1) * 512],
                         start=False, stop=True)
                S.activation(out=gsb[:, nb * 512:(nb + 1) * 512], in_=fps[:],
                             func=ActF.Gelu)
            gT = sp.tile([128, 8, 128], f32, tag="gT")
            for kt in range(8):
                pt = psum([128, 128], "tr")
                T.transpose(out=pt[:], in_=gsb[:, kt * 128:(kt + 1) * 128],
                            identity=id_sb[:])
                S.copy(out=gT[:, kt, :], in_=pt[:])
            o2 = psum([128, DM], "mm")
            for kt in range(8):
                T.matmul(o2[:], lhsT=gT[:, kt, :], rhs=w_sb["lin2_w"][:, kt, :],
                         start=(kt == 0), stop=(kt == 7))
            o_sb = layernorm(o2, hn, "ln2")

            n_out = min(128, QPC - q0)
            nc.sync.dma_start(out=out_d[q0:q0 + n_out], in_=o_sb[:n_out])

    nc.compile()
    return nc


def _prep_in_maps(inputs):
    src = np.asarray(inputs["src"], np.float32)
    ref = np.asarray(inputs["reference_points"], np.float32)
    qpe = np.asarray(inputs["query_pos_embed"], np.float32)

    consts, E3, REP, IDENT, scale_m = _host_consts()
    off_wx, off_wy = _perm_off_w(np.asarray(inputs["off_w"], np.float32))

    shared = dict(
        val_w=_ktiles(np.asarray(inputs["val_w"], np.float32)),
        off_wx=_ktiles(off_wx), off_wy=_ktiles(off_wy),
        aw_w=_ktiles(np.asarray(inputs["aw_w"], np.float32)),
        out_w=_ktiles(np.asarray(inputs["out_w"], np.float32)),
        lin1_w=_ktiles(np.asarray(inputs["lin1_w"], np.float32)),
        lin2_w=_ktiles(np.asarray(inputs["lin2_w"], np.float32)),
        consts=consts, e3=E3, rep=REP, ident=IDENT, scale_m=scale_m,
    )
    in_maps = []
    for core in range(NCORES):
        b, qh = core // 2, core % 2
        sl = slice(qh * QPC, (qh + 1) * QPC)
        src_q = np.zeros((QPAD, DM), np.float32)
        src_q[:QPC] = src[b, sl]
        qpe_q = np.zeros((QPAD, DM), np.float32)
        qpe_q[:QPC] = qpe[b, sl]
        refs = np.zeros((6, QPAD), np.float32)
        refs[0:3, :QPC] = ref[b, sl, :, 0].T
        refs[3:6, :QPC] = ref[b, sl, :, 1].T
        in_maps.append(dict(shared, src_full=np.ascontiguousarray(src[b]),
                            src_q=src_q, qpe_q=qpe_q, refs=refs))
    return in_maps


def kernel(**inputs):
    if "nc" not in _CACHE:
        _CACHE["nc"] = _build_program()
    nc = _CACHE["nc"]
    in_maps = _prep_in_maps(inputs)
    res = run_bass_kernel_spmd(nc, in_maps, core_ids=list(range(NCORES)))
    out = np.zeros((B, Lq, DM), np.float32)
    for core in range(NCORES):
        b, qh = core // 2, core % 2
        out[b, qh * QPC:(qh + 1) * QPC] = res.results[core]["out"]
    return out



# revision 3
# speedup vs baseline: 2.1518x; 2.1518x over previous
"""Trainium2 Bass kernel for DeformableTransformerEncoderLayer (v2).

Sharding: 8 cores = (batch b in 0..3) x (half of the 8400 queries).

v2 vs v1: the deformable gather is descriptor-count limited on HW, so each
bilinear sample is fetched with ONE 256B dma_gather descriptor instead of two:
stage 1 writes a row-packed bf16 table valcat where slot (h, parity, l, sy, xs)
holds [v[sy-1, x0], v[sy-1, x0+1], v[sy, x0], v[sy, x0+1]] interleaved as
[pp0, cp0, pp1, cp1] x 32 dims (x0 = 2*xs + parity).  Each level has H+1
slot-rows so the y-1/y row pair is always a single slot.  The combine tree
runs in bf16 (2x DVE) and query pos embed / transpose work moved to the host.
"""

import sys
import os
import numpy as np
from contextlib import ExitStack

for _p in ("/root/.axon_site/_ro/trn_rl_repo", "/opt/trn_rl_repo"):
    if os.path.isdir(_p) and _p not in sys.path:
        sys.path.insert(0, _p)

import concourse.bass as bass
import concourse.bacc as bacc
import concourse.tile as tile
from concourse import mybir
from concourse.bass_utils import run_bass_kernel_spmd

dt = mybir.dt
Alu = mybir.AluOpType
ActF = mybir.ActivationFunctionType
AX = mybir.AxisListType

# ---------------------------------------------------------------- problem dims
B, Lq, DM, NH, LVL, PTS, DFF, HD = 4, 8400, 256, 8, 3, 4, 1024, 32
SHAPES = [(80, 80), (40, 40), (20, 20)]          # (H, W)
LSI = [0, 6400, 8000]
NCORES = 8
QPC = Lq // 2                                     # queries per core = 4200
NBLK = 33
QPAD = NBLK * 128                                 # 4224
NJ = NH * LVL * PTS                               # 96 sample coords

# packed-slot table geometry (slots of 128 bf16 = 256B)
LVL_SLOTS = [(h + 1) * (w // 2) for h, w in SHAPES]     # 3240, 820, 210
LVLBASE = [0, LVL_SLOTS[0], LVL_SLOTS[0] + LVL_SLOTS[1]]  # 0, 3240, 4060
REG = sum(LVL_SLOTS)                              # 4270 slots per (head,parity)
NREG = NH * 2                                     # 16 regions
CAT_SLOTS = NREG * REG                            # 68320
HP_SLOTS = 4 * REG                                # 17080 slots per head-pair

_CACHE = {}


# ------------------------------------------------------------------ host prep
def _host_consts():
    j = np.arange(NJ)
    h = j // (LVL * PTS)
    l = (j % (LVL * PTS)) // PTS
    W = np.array([SHAPES[i][1] for i in range(LVL)], np.float32)[l]
    H = np.array([SHAPES[i][0] for i in range(LVL)], np.float32)[l]
    w2 = W / 2
    base = np.array(LVLBASE, np.float32)[l] + (h % 2) * (2 * REG) + w2
    consts = np.stack([W - 2, H - 1, w2, base], axis=1)
    consts = np.ascontiguousarray(consts.astype(np.float32))

    E3 = np.zeros((36, NJ), np.float32)
    E3[l, j] = 1.0
    E3[32 + l, j] = 1.0
    REP = np.zeros((16, 128), np.float32)
    REP[np.arange(128) % 16, np.arange(128)] = 1.0
    IDENT = np.eye(128, dtype=np.float32)
    scale_m = np.zeros((36, 1), np.float32)
    scale_m[0:3, 0] = [SHAPES[i][1] for i in range(LVL)]
    scale_m[32:35, 0] = [SHAPES[i][0] for i in range(LVL)]
    return consts, E3, REP, IDENT, scale_m


def _perm_off_w(off_w):
    cols = np.arange(NH * LVL * PTS * 2).reshape(NH, LVL, PTS, 2)
    return (np.ascontiguousarray(off_w[:, cols[..., 0].reshape(-1)]),
            np.ascontiguousarray(off_w[:, cols[..., 1].reshape(-1)]))


def _ktiles(w):
    K, N = w.shape
    return np.ascontiguousarray(w.reshape(K // 128, 128, N).astype(np.float32))


def _lvl_chunks(it, n):
    """split tile tokens [it*128, it*128+n) by level -> (a, b, l, ps0)."""
    t0, t1 = it * 128, it * 128 + n
    out = []
    for l in range(LVL):
        lo = LSI[l]
        hi = LSI[l] + SHAPES[l][0] * SHAPES[l][1]
        a, b = max(t0, lo), min(t1, hi)
        if a < b:
            out.append((a - t0, b - t0, l, a - lo))
    return out


# -------------------------------------------------------------- device program
def _build_program():
    nc = bacc.Bacc("TRN2", target_bir_lowering=False, debug=False, num_swdge_queues=4)
    f32 = dt.float32
    bf16 = dt.bfloat16

    src_full = nc.dram_tensor("src_full", [Lq, DM], f32, kind="ExternalInput")
    src_q = nc.dram_tensor("src_q", [QPAD, DM], f32, kind="ExternalInput")
    qT_q = nc.dram_tensor("qT_q", [2, 128, QPAD], f32, kind="ExternalInput")
    refs = nc.dram_tensor("refs", [6, QPAD], f32, kind="ExternalInput")
    w_in = {}
    for name, kt, n in (("val_w", 2, DM), ("off_wx", 2, NJ), ("off_wy", 2, NJ),
                        ("aw_w", 2, NJ), ("out_w", 2, DM), ("lin1_w", 2, DFF),
                        ("lin2_w", 8, DM)):
        w_in[name] = nc.dram_tensor(name, [kt, 128, n], f32, kind="ExternalInput")
    consts = nc.dram_tensor("consts", [NJ, 4], f32, kind="ExternalInput")
    e3 = nc.dram_tensor("e3", [36, NJ], f32, kind="ExternalInput")
    rep = nc.dram_tensor("rep", [16, 128], f32, kind="ExternalInput")
    ident = nc.dram_tensor("ident", [128, 128], f32, kind="ExternalInput")
    scale_m = nc.dram_tensor("scale_m", [36, 1], f32, kind="ExternalInput")

    out_d = nc.dram_tensor("out", [QPC, DM], f32, kind="ExternalOutput")
    valcat = nc.dram_tensor("valcat", [CAT_SLOTS, 128], bf16)

    def ap(base, off, dims):
        return bass.AP(tensor=base.tensor, offset=base.offset + off,
                       ap=[list(d) for d in dims])

    RSTRIDE = REG * 128          # region stride in elems
    HSTRIDE = 2 * RSTRIDE        # head stride in elems

    with tile.TileContext(nc) as tc, ExitStack() as ctx:
        V, S, T, G = nc.vector, nc.scalar, nc.tensor, nc.gpsimd

        def stt(out, in0, scalar, in1, op0, op1):
            return V.scalar_tensor_tensor(out=out, in0=in0, scalar=scalar,
                                          in1=in1, op0=op0, op1=op1)

        wp = ctx.enter_context(tc.tile_pool(name="weights", bufs=1))
        f32r = dt.float32r
        R_WEIGHTS = {"val_w", "out_w", "lin1_w", "lin2_w"}
        w_sb = {}
        for name, t in w_in.items():
            kt, n = t.shape[0], t.shape[2]
            wdt = f32r if name in R_WEIGHTS else f32
            s = wp.tile([128, kt, n], wdt, name=name + "_sb")
            src_ap = t[:].rearrange("a p n -> p a n")
            if name in R_WEIGHTS:
                src_ap = src_ap.bitcast(f32r)
            nc.sync.dma_start(out=s[:], in_=src_ap)
            w_sb[name] = s
        consts_sb = wp.tile([NJ, 4], f32)
        nc.sync.dma_start(out=consts_sb[:], in_=consts[:])
        e3_sb = wp.tile([36, NJ], f32)
        nc.sync.dma_start(out=e3_sb[:], in_=e3[:])
        rep_sb = wp.tile([16, 128], f32)
        nc.sync.dma_start(out=rep_sb[:], in_=rep[:])
        id_sb = wp.tile([128, 128], f32)
        nc.sync.dma_start(out=id_sb[:], in_=ident[:])
        scm_sb = wp.tile([36, 1], f32)
        nc.sync.dma_start(out=scm_sb[:], in_=scale_m[:])
        eps_sb = wp.tile([128, 1], f32)
        V.memset(eps_sb[:], 1e-5)
        zpad = wp.tile([16, 2560], bf16)
        V.memset(zpad[:], 0.0)
        # zero the never-data-written table bands: per level, slot-row 0 prev
        # positions and slot-row H cur positions (weights there are 0, but the
        # gathered payload must be finite).
        for l in range(LVL):
            Hl, Wl = SHAPES[l]
            nc.sync.dma_start(
                out=ap(valcat[:], LVLBASE[l] * 128,
                       [[RSTRIDE, NREG], [64, Wl], [1, 32]]),
                in_=zpad[:16, :Wl * 32].rearrange("p (a b) -> p a b", b=32))
            nc.sync.dma_start(
                out=ap(valcat[:], LVLBASE[l] * 128 + Hl * Wl * 64 + 32,
                       [[RSTRIDE, NREG], [64, Wl], [1, 32]]),
                in_=zpad[:16, :Wl * 32].rearrange("p (a b) -> p a b", b=32))
            # parity-1 regions have two 32-elem never-written holes per level
            for hole in (( Hl * Wl - 1) * 64 + 32, ((Hl + 1) * Wl - 1) * 64):
                nc.sync.dma_start(
                    out=ap(valcat[:], RSTRIDE + LVLBASE[l] * 128 + hole,
                           [[2 * RSTRIDE, 8], [1, 32]]),
                    in_=zpad[:8, :32])

        cWm2 = consts_sb[:, 0:1]
        cHm1 = consts_sb[:, 1:2]
        cW2 = consts_sb[:, 2:3]
        cBase = consts_sb[:, 3:4]

        pps = ctx.enter_context(tc.tile_pool(name="pps", bufs=2, space="PSUM"))

        def psum(shape, tag):
            return pps.tile(shape, f32, tag=tag, name=tag)

        # ------------------------------------------------------------ stage 1
        with tc.tile_pool(name="s1", bufs=2) as s1:
            NT1 = (Lq + 127) // 128  # 66
            for it in range(NT1):
                n = min(128, Lq - it * 128)
                st = s1.tile([128, DM], f32, tag="st")
                nc.sync.dma_start(out=st[:n], in_=src_full[it * 128: it * 128 + n])
                sT = s1.tile([128, 2, 128], f32r, tag="sT")
                for kt in range(2):
                    pt = psum([128, 128], "tr")
                    T.transpose(out=pt[:, :n], in_=st[:n, kt * 128:(kt + 1) * 128],
                                identity=id_sb[:n, :n])
                    S.copy(out=sT[:, kt, :n], in_=pt[:, :n])
                vps = psum([128, DM], "mm")
                T.matmul(vps[:n], lhsT=sT[:, 0, :n],
                         rhs=w_sb["val_w"][:, 0, :],
                         start=True, stop=False)
                T.matmul(vps[:n], lhsT=sT[:, 1, :n],
                         rhs=w_sb["val_w"][:, 1, :],
                         start=False, stop=True)
                vsb = s1.tile([128, DM], bf16, tag="vsb")
                S.copy(out=vsb[:n], in_=vps[:n])
                for a, b, l, ps0 in _lvl_chunks(it, n):
                    Wl = SHAPES[l][1]
                    base_l = LVLBASE[l] * 128
                    for p in (0, 1):
                        # cur copy: row y at slot-row y, +32 within 64-block
                        aa, pss = a, ps0
                        if p == 1 and ps0 == 0:
                            aa, pss = a + 1, 1
                        if aa < b:
                            nc.sync.dma_start(
                                out=ap(valcat[:],
                                       p * RSTRIDE + base_l + (pss - p) * 64 + 32,
                                       [[64, b - aa], [HSTRIDE, 8], [1, 32]]),
                                in_=vsb[aa:b].rearrange("t (h d) -> t h d", h=8))
                        # prev copy: row y at slot-row y+1, +0 within 64-block
                        nc.sync.dma_start(
                            out=ap(valcat[:],
                                   p * RSTRIDE + base_l + (ps0 + Wl - p) * 64,
                                   [[64, b - a], [HSTRIDE, 8], [1, 32]]),
                            in_=vsb[a:b].rearrange("t (h d) -> t h d", h=8))

        # ------------------------------------------------------------ stage 2
        sp = ctx.enter_context(tc.tile_pool(name="sp", bufs=2))
        sc = ctx.enter_context(tc.tile_pool(name="sc", bufs=1))
        mp = ctx.enter_context(tc.tile_pool(name="mp", bufs=2))
        gp = ctx.enter_context(tc.tile_pool(name="gp", bufs=4))
        gf = ctx.enter_context(tc.tile_pool(name="gf", bufs=2))

        RNE = 12582912.0  # 1.5 * 2^23
        J = NJ

        for ib in range(NBLK):
            q0 = ib * 128
            sq = sp.tile([128, DM], f32, tag="sq")
            nc.sync.dma_start(out=sq[:], in_=src_q[q0:q0 + 128])
            qT = sp.tile([128, 2, 128], f32, tag="qT")
            for kt in range(2):
                nc.sync.dma_start(out=qT[:, kt, :], in_=qT_q[kt, :, q0:q0 + 128])
            rf = sc.tile([36, 128], f32, tag="rf")
            nc.sync.dma_start(out=rf[0:3], in_=refs[0:3, q0:q0 + 128])
            nc.sync.dma_start(out=rf[32:35], in_=refs[3:6, q0:q0 + 128])
            rw = sc.tile([36, 128], f32, tag="rw")
            V.tensor_scalar(out=rw[0:3], in0=rf[0:3], scalar1=scm_sb[0:3],
                            scalar2=-0.5, op0=Alu.mult, op1=Alu.add)
            V.tensor_scalar(out=rw[32:35], in0=rf[32:35], scalar1=scm_sb[32:35],
                            scalar2=-0.5, op0=Alu.mult, op1=Alu.add)

            xy = {}
            for name, wkey, r0, r1 in (("x", "off_wx", 0, 3), ("y", "off_wy", 32, 35)):
                pxy = psum([128, 128], "mm2")
                T.matmul(pxy[:J], lhsT=w_sb[wkey][:, 0, :], rhs=qT[:, 0, :],
                         start=True, stop=False)
                T.matmul(pxy[:J], lhsT=w_sb[wkey][:, 1, :], rhs=qT[:, 1, :],
                         start=False, stop=False)
                T.matmul(pxy[:J], lhsT=e3_sb[r0:r1, :], rhs=rw[r0:r1, :],
                         start=False, stop=True)
                xs = sc.tile([128, 128], f32, tag="xy" + name)
                S.copy(out=xs[:J], in_=pxy[:J])
                xy[name] = xs
            x_sb, y_sb = xy["x"], xy["y"]

            def nt(tag):
                return sc.tile([128, 128], f32, tag=tag, name=tag)

            def floor_(src_t, tag):
                a = nt(tag + "a")
                V.tensor_scalar(out=a[:J], in0=src_t[:J], scalar1=RNE,
                                scalar2=-RNE, op0=Alu.add, op1=Alu.add)
                g_ = nt(tag + "g")
                stt(g_[:J], a[:J], 0.0, src_t[:J], Alu.bypass, Alu.is_gt)
                f = nt(tag + "f")
                stt(f[:J], a[:J], 0.0, g_[:J], Alu.bypass, Alu.subtract)
                return f

            x0 = floor_(x_sb, "x0")
            fx = nt("fx")
            stt(fx[:J], x_sb[:J], 0.0, x0[:J], Alu.bypass, Alu.subtract)
            y0 = floor_(y_sb, "y0")
            fy = nt("fy")
            stt(fy[:J], y_sb[:J], 0.0, y0[:J], Alu.bypass, Alu.subtract)

            xc = nt("xc")
            V.tensor_scalar(out=xc[:J], in0=x0[:J], scalar1=0.0, scalar2=cWm2,
                            op0=Alu.max, op1=Alu.min)
            eqA0 = nt("eqA0")
            stt(eqA0[:J], x0[:J], 0.0, xc[:J], Alu.bypass, Alu.is_equal)
            eqA1 = nt("eqA1")
            stt(eqA1[:J], x0[:J], 1.0, xc[:J], Alu.add, Alu.is_equal)
            eqB0 = nt("eqB0")
            stt(eqB0[:J], x0[:J], -1.0, xc[:J], Alu.add, Alu.is_equal)
            dA = nt("dA")
            stt(dA[:J], eqA1[:J], 0.0, eqA0[:J], Alu.bypass, Alu.subtract)
            wA = nt("wA")
            stt(wA[:J], fx[:J], 0.0, dA[:J], Alu.bypass, Alu.mult)
            stt(wA[:J], wA[:J], 0.0, eqA0[:J], Alu.bypass, Alu.add)
            dB = nt("dB")
            stt(dB[:J], eqA0[:J], 0.0, eqB0[:J], Alu.bypass, Alu.subtract)
            wB = nt("wB")
            stt(wB[:J], fx[:J], 0.0, dB[:J], Alu.bypass, Alu.mult)
            stt(wB[:J], wB[:J], 0.0, eqB0[:J], Alu.bypass, Alu.add)

            yr0 = nt("yr0")
            V.tensor_scalar(out=yr0[:J], in0=y0[:J], scalar1=0.0, scalar2=cHm1,
                            op0=Alu.max, op1=Alu.min)
            vy0 = nt("vy0")
            stt(vy0[:J], y0[:J], 0.0, yr0[:J], Alu.bypass, Alu.is_equal)
            wy0 = nt("wy0")
            stt(wy0[:J], fy[:J], 0.0, vy0[:J], Alu.bypass, Alu.mult)
            stt(wy0[:J], vy0[:J], 0.0, wy0[:J], Alu.bypass, Alu.subtract)
            y1 = nt("y1")
            V.tensor_scalar(out=y1[:J], in0=y0[:J], scalar1=1.0, op0=Alu.add,
                            scalar2=None)
            yr1 = nt("yr1")
            V.tensor_scalar(out=yr1[:J], in0=y1[:J], scalar1=0.0, scalar2=cHm1,
                            op0=Alu.max, op1=Alu.min)
            vy1 = nt("vy1")
            stt(vy1[:J], y1[:J], 0.0, yr1[:J], Alu.bypass, Alu.is_equal)
            wy1 = nt("wy1")
            stt(wy1[:J], fy[:J], 0.0, vy1[:J], Alu.bypass, Alu.mult)

            # slot index: sl = clamp(y0,-1,H-1)*W2 + floor(xc/2) + par*REG + base
            tcl = nt("tcl")
            V.tensor_scalar(out=tcl[:J], in0=y0[:J], scalar1=-1.0, scalar2=cHm1,
                            op0=Alu.max, op1=Alu.min)
            xh2 = nt("xh2")
            V.tensor_scalar(out=xh2[:J], in0=xc[:J], scalar1=0.5, op0=Alu.mult,
                            scalar2=None)
            xh = floor_(xh2, "xh")
            par = nt("par")
            stt(par[:J], xh[:J], -2.0, xc[:J], Alu.mult, Alu.add)
            sl = mp.tile([128, 128], f32, tag="sl", name="sl")
            V.tensor_scalar(out=sl[:J], in0=tcl[:J], scalar1=cW2, scalar2=cBase,
                            op0=Alu.mult, op1=Alu.add)
            stt(sl[:J], sl[:J], 0.0, xh[:J], Alu.bypass, Alu.add)
            stt(sl[:J], par[:J], float(REG), sl[:J], Alu.mult, Alu.add)

            # wrapped idx [16, 768] -> replicate to [128, 768] -> int16
            wf16 = mp.tile([16, 768], f32, tag="wf16")
            for qb in range(8):
                pt = psum([128, 128], "tr")
                T.transpose(out=pt[:16, :J], in_=sl[:J, qb * 16:(qb + 1) * 16],
                            identity=id_sb[:J, :J])
                dst = ap(wf16[:], qb, [[768, 16], [8, 96]])
                V.tensor_copy(out=dst, in_=pt[:16, :J])
            idxw = mp.tile([128, 768], dt.int16, tag="idxw")
            for seg, c0, c1 in ((0, 0, 512), (1, 512, 768)):
                pr = psum([128, 512], "w512")
                T.matmul(pr[:, :c1 - c0], lhsT=rep_sb[:], rhs=wf16[:, c0:c1],
                         start=True, stop=True)
                V.tensor_copy(out=idxw[:, c0:c1], in_=pr[:, :c1 - c0])

            # attention weight softmax
            awp = psum([128, 128], "mm2")
            T.matmul(awp[:, :J], lhsT=qT[:, 0, :], rhs=w_sb["aw_w"][:, 0, :],
                     start=True, stop=False)
            T.matmul(awp[:, :J], lhsT=qT[:, 1, :], rhs=w_sb["aw_w"][:, 1, :],
                     start=False, stop=True)
            aw_sb = sc.tile([128, NJ], f32, tag="aw_sb")
            S.copy(out=aw_sb[:], in_=awp[:, :J])
            rmax = sc.tile([128, 8], f32, tag="rmax")
            V.tensor_reduce(out=rmax[:],
                            in_=aw_sb[:].rearrange("p (h m) -> p h m", h=8),
                            axis=AX.X, op=Alu.max)
            xm = sc.tile([128, NJ], f32, tag="xm")
            stt(xm[:], aw_sb[:], 0.0,
                ap(rmax[:], 0, [[8, 128], [1, 8], [0, 12]]),
                Alu.bypass, Alu.subtract)
            exw = sc.tile([128, NJ], f32, tag="exw")
            S.activation(out=exw[:], in_=xm[:], func=ActF.Exp)
            ssum = sc.tile([128, 8], f32, tag="ssum")
            V.tensor_reduce(out=ssum[:],
                            in_=exw[:].rearrange("p (h m) -> p h m", h=8),
                            axis=AX.X, op=Alu.add)
            rec = sc.tile([128, 8], f32, tag="rec")
            V.reciprocal(out=rec[:], in_=ssum[:])
            asm = mp.tile([128, NJ], f32, tag="asm")
            stt(asm[:], exw[:], 0.0,
                ap(rec[:], 0, [[8, 128], [1, 8], [0, 12]]),
                Alu.bypass, Alu.mult)

            # token-major corner weights: wfull[:, j*4 + c*2 + r] = a*wy_r*wx_c
            wxp = mp.tile([128, 192], f32, tag="wxp")
            wyp = mp.tile([128, 192], f32, tag="wyp")
            for src_t, dstt, col in ((wA, wxp, 0), (wB, wxp, 96),
                                     (wy0, wyp, 0), (wy1, wyp, 96)):
                pt = psum([128, 128], "tr")
                T.transpose(out=pt[:, :J], in_=src_t[:J, :],
                            identity=id_sb[:J, :J])
                S.copy(out=dstt[:, col:col + 96], in_=pt[:, :J])
            wrow = mp.tile([128, 192], f32, tag="wrow")
            stt(wrow[:], wyp[:], 0.0,
                ap(asm[:], 0, [[96, 128], [0, 2], [1, 96]]), Alu.bypass, Alu.mult)
            wfull = mp.tile([128, 384], f32, tag="wfull")
            for rr in (0, 1):
                stt(ap(wfull[:], rr, [[384, 128], [4, 96], [2, 2]]),
                    ap(wrow[:], rr * 96, [[192, 128], [1, 96], [0, 2]]),
                    0.0,
                    ap(wxp[:], 0, [[192, 128], [1, 96], [96, 2]]),
                    Alu.bypass, Alu.mult)

            # gather + combine per head-pair
            att = mp.tile([128, DM], f32, tag="att")
            for hp in range(4):
                g = gp.tile([128, 24 * 128], bf16, tag="g")
                G.dma_gather(
                    out_ap=g[:].rearrange("p (j e) -> p j e", e=128),
                    in_ap=ap(valcat[:], hp * HP_SLOTS * 128,
                             [[128, HP_SLOTS], [1, 128]]),
                    idxs_ap=idxw[:, hp * 192:(hp + 1) * 192],
                    num_idxs=3072, num_idxs_reg=3072,
                    elem_size=128, elem_step=128, single_packet=False,
                    queue_num=hp)
                wexp = gp.tile([128, 24 * 128], bf16, tag="wexp")
                S.copy(out=wexp[:].rearrange("p (u d) -> p u d", d=32),
                       in_=ap(wfull[:], hp * 96, [[384, 128], [1, 96], [0, 32]]))
                wg = gp.tile([128, 24 * 128], bf16, tag="wg")
                V.tensor_tensor(out=wg[:], in0=g[:], in1=wexp[:], op=Alu.mult)
                f1 = gf.tile([128, 1536], bf16, tag="f1")
                V.tensor_tensor(out=f1[:],
                    in0=ap(wg[:], 0, [[3072, 128], [64, 48], [1, 32]]),
                    in1=ap(wg[:], 32, [[3072, 128], [64, 48], [1, 32]]),
                    op=Alu.add)
                f2 = gf.tile([128, 768], bf16, tag="f2")
                V.tensor_tensor(out=f2[:],
                    in0=ap(f1[:], 0, [[1536, 128], [64, 24], [1, 32]]),
                    in1=ap(f1[:], 32, [[1536, 128], [64, 24], [1, 32]]),
                    op=Alu.add)
                f3 = gf.tile([128, 384], bf16, tag="f3")
                V.tensor_tensor(out=f3[:],
                    in0=ap(f2[:], 0, [[768, 128], [384, 2], [64, 6], [1, 32]]),
                    in1=ap(f2[:], 32, [[768, 128], [384, 2], [64, 6], [1, 32]]),
                    op=Alu.add)
                f4 = gf.tile([128, 192], bf16, tag="f4")
                V.tensor_tensor(out=f4[:],
                    in0=ap(f3[:], 0, [[384, 128], [192, 2], [64, 3], [1, 32]]),
                    in1=ap(f3[:], 32, [[384, 128], [192, 2], [64, 3], [1, 32]]),
                    op=Alu.add)
                t5 = gf.tile([128, 64], f32, tag="t5")
                V.tensor_tensor(out=t5[:].rearrange("p (b d) -> p b d", d=32),
                    in0=ap(f4[:], 0, [[192, 128], [96, 2], [1, 32]]),
                    in1=ap(f4[:], 32, [[192, 128], [96, 2], [1, 32]]),
                    op=Alu.add)
                V.tensor_tensor(out=att[:, hp * 64:(hp + 1) * 64].rearrange(
                        "p (b d) -> p b d", d=32),
                    in0=t5[:].rearrange("p (b d) -> p b d", d=32),
                    in1=ap(f4[:], 64, [[192, 128], [96, 2], [1, 32]]),
                    op=Alu.add)

            # out-proj + residual + LN1
            aT = sp.tile([128, 2, 128], f32r, tag="aT")
            for kt in range(2):
                pt = psum([128, 128], "tr")
                T.transpose(out=pt[:], in_=att[:, kt * 128:(kt + 1) * 128],
                            identity=id_sb[:])
                S.copy(out=aT[:, kt, :], in_=pt[:])
            ops_ = psum([128, DM], "mm")
            T.matmul(ops_[:], lhsT=aT[:, 0, :],
                     rhs=w_sb["out_w"][:, 0, :],
                     start=True, stop=False)
            T.matmul(ops_[:], lhsT=aT[:, 1, :],
                     rhs=w_sb["out_w"][:, 1, :],
                     start=False, stop=True)

            def layernorm(src_ps, res_sb, tag):
                h1 = sc.tile([128, DM], f32, tag=tag + "h1")
                stt(h1[:], src_ps[:], 0.0, res_sb[:], Alu.bypass, Alu.add)
                mr = sc.tile([128, 1], f32, tag=tag + "mr")
                V.tensor_reduce(out=mr[:], in_=h1[:], axis=AX.X, op=Alu.add)
                m = sc.tile([128, 1], f32, tag=tag + "m")
                V.tensor_scalar(out=m[:], in0=mr[:], scalar1=1.0 / DM,
                                op0=Alu.mult, scalar2=None)
                d = sc.tile([128, DM], f32, tag=tag + "d")
                stt(d[:], h1[:], 0.0, ap(m[:], 0, [[1, 128], [0, DM]]),
                    Alu.bypass, Alu.subtract)
                sq2 = sc.tile([128, DM], f32, tag=tag + "sq")
                S.activation(out=sq2[:], in_=d[:], func=ActF.Square)
                vr = sc.tile([128, 1], f32, tag=tag + "vr")
                V.tensor_reduce(out=vr[:], in_=sq2[:], axis=AX.X, op=Alu.add)
                sd = sc.tile([128, 1], f32, tag=tag + "sd")
                S.activation(out=sd[:], in_=vr[:], func=ActF.Sqrt,
                             scale=1.0 / DM, bias=eps_sb[:])
                rstd = sc.tile([128, 1], f32, tag=tag + "rs")
                V.reciprocal(out=rstd[:], in_=sd[:])
                o = sp.tile([128, DM], f32, tag=tag + "o")
                V.tensor_scalar(out=o[:], in0=d[:], scalar1=rstd[:],
                                op0=Alu.mult, scalar2=None)
                return o

            hn = layernorm(ops_, sq, "ln1")

            hT = sp.tile([128, 2, 128], f32r, tag="hT")
            for kt in range(2):
                pt = psum([128, 128], "tr")
                T.transpose(out=pt[:], in_=hn[:, kt * 128:(kt + 1) * 128],
                            identity=id_sb[:])
                S.copy(out=hT[:, kt, :], in_=pt[:])
            gsb = sp.tile([128, DFF], f32, tag="gsb")
            for nb in range(2):
                fps = psum([128, 512], "w512")
                T.matmul(fps[:], lhsT=hT[:, 0, :],
                         rhs=w_sb["lin1_w"][:, 0, nb * 512:(nb + 1) * 512],
                         start=True, stop=False)
                T.matmul(fps[:], lhsT=hT[:, 1, :],
                         rhs=w_sb["lin1_w"][:, 1, nb * 512:(nb + 1) * 512],
                         start=False, stop=True)
                S.activation(out=gsb[:, nb * 512:(nb + 1) * 512], in_=fps[:],
                             func=ActF.Gelu)
            gT = sp.tile([128, 8, 128], f32r, tag="gT")
            for kt in range(8):
                pt = psum([128, 128], "tr")
                T.transpose(out=pt[:], in_=gsb[:, kt * 128:(kt + 1) * 128],
                            identity=id_sb[:])
                S.copy(out=gT[:, kt, :], in_=pt[:])
            o2 = psum([128, DM], "mm")
            for kt in range(8):
                T.matmul(o2[:], lhsT=gT[:, kt, :],
                         rhs=w_sb["lin2_w"][:, kt, :],
                         start=(kt == 0), stop=(kt == 7))
            o_sb = layernorm(o2, hn, "ln2")

            n_out = min(128, QPC - q0)
            nc.sync.dma_start(out=out_d[q0:q0 + n_out], in_=o_sb[:n_out])

    nc.compile()
    return nc


def _prep_in_maps(inputs):
    src = np.asarray(inputs["src"], np.float32)
    ref = np.asarray(inputs["reference_points"], np.float32)
    qpe = np.asarray(inputs["query_pos_embed"], np.float32)

    consts, E3, REP, IDENT, scale_m = _host_consts()
    off_wx, off_wy = _perm_off_w(np.asarray(inputs["off_w"], np.float32))

    shared = dict(
        val_w=_ktiles(np.asarray(inputs["val_w"], np.float32)),
        off_wx=_ktiles(off_wx), off_wy=_ktiles(off_wy),
        aw_w=_ktiles(np.asarray(inputs["aw_w"], np.float32)),
        out_w=_ktiles(np.asarray(inputs["out_w"], np.float32)),
        lin1_w=_ktiles(np.asarray(inputs["lin1_w"], np.float32)),
        lin2_w=_ktiles(np.asarray(inputs["lin2_w"], np.float32)),
        consts=consts, e3=E3, rep=REP, ident=IDENT, scale_m=scale_m,
    )
    q_full = src + qpe
    in_maps = []
    for core in range(NCORES):
        b, qh = core // 2, core % 2
        sl = slice(qh * QPC, (qh + 1) * QPC)
        src_q = np.zeros((QPAD, DM), np.float32)
        src_q[:QPC] = src[b, sl]
        qT_q = np.zeros((2, 128, QPAD), np.float32)
        qT_q[:, :, :QPC] = q_full[b, sl].T.reshape(2, 128, QPC)
        refs = np.zeros((6, QPAD), np.float32)
        refs[0:3, :QPC] = ref[b, sl, :, 0].T
        refs[3:6, :QPC] = ref[b, sl, :, 1].T
        in_maps.append(dict(shared, src_full=np.ascontiguousarray(src[b]),
                            src_q=src_q, qT_q=np.ascontiguousarray(qT_q),
                            refs=refs))
    return in_maps


def kernel(**inputs):
    if "nc" not in _CACHE:
        _CACHE["nc"] = _build_program()
    nc = _CACHE["nc"]
    in_maps = _prep_in_maps(inputs)
    res = run_bass_kernel_spmd(nc, in_maps, core_ids=list(range(NCORES)))
    out = np.zeros((B, Lq, DM), np.float32)
    for core in range(NCORES):
        b, qh = core // 2, core % 2
        out[b, qh * QPC:(qh + 1) * QPC] = res.results[core]["out"]
    return out
